# revision 15
# baseline (speedup 1.0000x reference)
"""EventAttention Trainium2 kernel (8 NeuronCores, SPMD + collectives).

Strategy (v2):
  - Shard the N=20000 points across 8 cores (2500 each). Each core builds
    the gather tables ONLY for its own shard from its own (bf16) feature
    slice, then three on-device AllGathers replicate the full tables:
      T_L   [20480, 384]  (kL|vL|uL rows, f32)   <- AG of per-core [2560,384]
      T_KVG [20480, 256]  (kG|vG rows, f32)      <- AG of per-core [2560,256]
      T_G   [3072, 384]   (kmax|vmax|uGd, f32)   <- AG of per-core [384,384]
    This removes the 20.6MB/core replicated feature upload of v1.
  - The downsampled M=2500 set is sharded 320/core (padded to 384 rows);
    each core max-pools k/v for its block, then T_G is all-gathered before
    the inv_pair_idx gather (as per the sharding hint).
  - Row remapping (host side): point p -> (p//2500)*2560 + p%2500,
    down-point m -> (m//320)*384 + m%320, so AllGather's axis-0 block
    concatenation lines up with gather indices.
  - Uploads are minimized: features + large weights in bf16, indices as
    compact [16, n] int16 (replicated to 128 partitions on device), output
    downloaded in bf16. ~2.3MB/core up + 1.3MB/core down vs ~25MB/core up
    in v1.
  - The runner caches the jitted shard_map callable (run_bass_kernel_spmd
    re-traces and re-runs the NEFF compile check every call; we only pay
    that once).
  - Attention math is unchanged from v1: token-rows layout [128 partitions,
    16 slots, 128 ch]; LN via per-slot bn_stats; softmax-over-K via strided
    reduces; pe-MLP layer 2 via per-slot transpose + matmul with the q-row
    folded into PSUM through an identity matmul.

Relies on the spec-guaranteed fills: all *_b biases zero, fc_g ones,
fc_b zeros (asserted at runtime).
"""
import sys
import numpy as np

sys.path.insert(0, "/opt/trn_rl_repo")

N, K, A, DIM, M = 20000, 16, 128, 256, 2500
NCORES = 8
NPC = N // NCORES                     # 2500 points per core
PC_CH = 20                            # chunks per core
PCPAD = PC_CH * 128                   # 2560
NFULL = NCORES * PCPAD                # 20480 rows in gathered tables
MPC = 320                             # down-points per core
MB_CH = 3                             # chunks per core for M block
MBPAD = MB_CH * 128                   # 384
MFULL = NCORES * MBPAD                # 3072 rows in gathered T_G
SCALE = float(np.sqrt(A))
EPS = 1e-5
GS = 16                               # slots per dma_gather call (=K)

# ES (f32 [4, 3200]) column layout
ES_EV = 0            # evT_own   [4, 2560]
ES_DEV = PCPAD       # devT_own  [4, 384]
ES_WU = PCPAD + MBPAD            # wu = [w1L|w1G]  [4, 256]
ES_W1G = ES_WU + 128             # w1G alone = second half of wu
ES_COLS = ES_WU + 256            # 3200

# IDX (i16 [16, 5504]) column layout
IX_L = 0                         # lidx  [16, 2560]
IX_G = PCPAD                     # gidx  [16, 2560]
IX_P = 2 * PCPAD                 # pidx  [16, 384]
IX_COLS = 2 * PCPAD + MBPAD      # 5504

# WB (bf16 [128, 2560]) column layout
WB_KV0, WB_KV1 = 0, 512          # wkv0/wkv1 [128, 512] each
WB_Q0, WB_Q1 = 1024, 1280        # wq0/wq1 [128, 256] each
WB_P1A, WB_P1B = 1536, 1792      # proj w1 halves [128, 256]
WB_P2A, WB_P2B = 2048, 2304      # proj w2 halves [128, 256]
WB_COLS = 2560

_CACHE = {}


def _build():
    import concourse.bacc as bacc
    import concourse.tile as tile
    from contextlib import ExitStack
    import concourse.bass as bass
    from concourse import mybir
    from concourse.masks import make_identity

    f32 = mybir.dt.float32
    bf16 = mybir.dt.bfloat16
    i16 = mybir.dt.int16
    Alu = mybir.AluOpType
    Act = mybir.ActivationFunctionType
    AxX = mybir.AxisListType.X

    def bcast_mid(ap2d, count):
        ap = ap2d.ap
        assert len(ap) == 2
        return bass.AP(ap2d.tensor, ap2d.offset,
                       [list(ap[0]), [0, count], list(ap[1])])

    nc = bacc.Bacc("TRN2", target_bir_lowering=False, debug=False,
                   num_devices=NCORES)

    featR = nc.dram_tensor("featR", [PCPAD, DIM], bf16, kind="ExternalInput")
    ES = nc.dram_tensor("ES", [4, ES_COLS], f32, kind="ExternalInput")
    IDX = nc.dram_tensor("IDX", [16, IX_COLS], i16, kind="ExternalInput")
    # weights arrive row-sharded (16 rows per core) and are all-gathered
    WBs = nc.dram_tensor("WBs", [16, WB_COLS], bf16, kind="ExternalInput")
    WFs = nc.dram_tensor("WFs", [16, 256], f32, kind="ExternalInput")
    out_d = nc.dram_tensor("out", [PCPAD, DIM], bf16, kind="ExternalOutput")

    RG = [list(range(NCORES))]

    with tile.TileContext(nc) as tc, ExitStack() as ctx:
        # ---------------- persistent SBUF ----------------
        pers = ctx.enter_context(tc.tile_pool(name="pers", bufs=1))
        dram = ctx.enter_context(tc.tile_pool(name="dram", bufs=1,
                                              space="DRAM"))

        ident = pers.tile([128, 128], f32)
        make_identity(nc, ident[:])
        eps_t = pers.tile([128, 1], f32)
        nc.vector.memset(eps_t[:], EPS)

        wb_bi = dram.tile([16, WB_COLS], bf16, tag="wb_bi")
        wb_bo = dram.tile([128, WB_COLS], bf16, tag="wb_bo",
                          addr_space="Shared")
        nc.sync.dma_start(wb_bi[:], WBs[:, :])
        nc.gpsimd.collective_compute(
            "AllGather", mybir.AluOpType.bypass, replica_groups=RG,
            ins=[wb_bi.opt()], outs=[wb_bo.opt()])
        wb_t = pers.tile([128, WB_COLS], bf16, tag="wb")
        nc.sync.dma_start(wb_t[:], wb_bo[:, :])
        wf_bi = dram.tile([16, 256], f32, tag="wf_bi")
        wf_bo = dram.tile([128, 256], f32, tag="wf_bo", addr_space="Shared")
        nc.sync.dma_start(wf_bi[:], WFs[:, :])
        nc.gpsimd.collective_compute(
            "AllGather", mybir.AluOpType.bypass, replica_groups=RG,
            ins=[wf_bi.opt()], outs=[wf_bo.opt()])
        wf_t = pers.tile([128, 256], f32, tag="wf")
        nc.sync.dma_start(wf_t[:], wf_bo[:, :])
        es_t = pers.tile([4, ES_COLS], f32, tag="es")
        nc.sync.dma_start(es_t[:], ES[:, :])
        idx_t = pers.tile([128, IX_COLS], i16, tag="idx")
        for a in range(8):
            nc.sync.dma_start(idx_t[16 * a:16 * (a + 1), :], IDX[:, :])

        qL_own = pers.tile([128, PCPAD], f32, tag="qL_own")
        qG_own = pers.tile([128, PCPAD], f32, tag="qG_own")
        uL_own = pers.tile([128, PCPAD], f32, tag="uL_own")
        uG_own = pers.tile([128, PCPAD], f32, tag="uG_own")
        la_all = pers.tile([128, PCPAD], f32, tag="la_all")

        # local DRAM table shards + all-gathered tables
        T_L_own = dram.tile([PCPAD, 384], f32, tag="T_L_own")
        T_KVG_own = dram.tile([PCPAD, 256], f32, tag="T_KVG_own")
        T_G_own = dram.tile([MBPAD, 384], f32, tag="T_G_own")
        T_L = dram.tile([NFULL, 384], f32, tag="T_L", addr_space="Shared")
        T_KVG = dram.tile([NFULL, 256], f32, tag="T_KVG",
                          addr_space="Shared")
        T_G = dram.tile([MFULL, 384], f32, tag="T_G", addr_space="Shared")

        # ---------------- phase A: own-shard q/u + tables ----------------
        with ExitStack() as pa:
            sba = pa.enter_context(tc.tile_pool(name="sba", bufs=3))
            sbf = pa.enter_context(tc.tile_pool(name="sbf", bufs=1))
            psa = pa.enter_context(tc.tile_pool(name="psa", bufs=2,
                                                space="PSUM"))
            # transpose the row-major feature shard on device: one
            # dma_gather(transpose=True) with an iota index delivers
            # [128 ch, 2 groups, 2560 points] directly.
            fidx = sbf.tile([128, PCPAD // 16], i16, tag="fidx")
            nc.gpsimd.iota(fidx[0:16, :], pattern=[[16, PCPAD // 16]],
                           base=0, channel_multiplier=1)
            for a in range(1, 8):
                nc.sync.dma_start(fidx[16 * a:16 * (a + 1), :], fidx[0:16, :])
            fT = sbf.tile([128, 2, PCPAD], bf16, tag="fT")
            nc.gpsimd.dma_gather(fT[:], featR[:], fidx[:], PCPAD, PCPAD,
                                 DIM, transpose=True, single_packet=False)
            for c in range(PC_CH):
                sl = slice(c * 128, (c + 1) * 128)
                ft0 = fT[:, 0, sl]
                ft1 = fT[:, 1, sl]
                psq = psa.tile([128, 256], f32, tag="psq")
                nc.tensor.matmul(psq[:], lhsT=ft0,
                                 rhs=wb_t[:, WB_Q0:WB_Q0 + 256],
                                 start=True, stop=False)
                nc.tensor.matmul(psq[:], lhsT=ft1,
                                 rhs=wb_t[:, WB_Q1:WB_Q1 + 256],
                                 start=False, stop=True)
                pskv = psa.tile([128, 512], f32, tag="pskv")
                nc.tensor.matmul(pskv[:], lhsT=ft0,
                                 rhs=wb_t[:, WB_KV0:WB_KV0 + 512],
                                 start=True, stop=False)
                nc.tensor.matmul(pskv[:], lhsT=ft1,
                                 rhs=wb_t[:, WB_KV1:WB_KV1 + 512],
                                 start=False, stop=True)
                psu = psa.tile([128, 256], f32, tag="psu")
                nc.tensor.matmul(psu[:], lhsT=es_t[:, sl],
                                 rhs=es_t[:, ES_WU:ES_WU + 256],
                                 start=True, stop=True)
                nc.vector.tensor_copy(qL_own[:, sl], psq[:, 0:128])
                nc.scalar.copy(qG_own[:, sl], psq[:, 128:256])
                nc.vector.tensor_copy(uL_own[:, sl], psu[:, 0:128])
                nc.scalar.copy(uG_own[:, sl], psu[:, 128:256])
                stg = sba.tile([128, 640], f32, tag="stg")
                nc.scalar.copy(stg[:, 0:256], pskv[:, 0:256])      # kL|vL
                nc.vector.tensor_copy(stg[:, 256:384], psu[:, 0:128])  # uL
                nc.vector.tensor_copy(stg[:, 384:640], pskv[:, 256:512])
                nc.sync.dma_start(T_L_own[sl, :], stg[:, 0:384])
                nc.sync.dma_start(T_KVG_own[sl, :], stg[:, 384:640])

            # A3: down-point u table (global pe layer-1 on down events)
            for c in range(MB_CH):
                sl = slice(c * 128, (c + 1) * 128)
                psd = psa.tile([128, 128], f32, tag="psu")
                nc.tensor.matmul(psd[:],
                                 lhsT=es_t[:, ES_DEV + c * 128:
                                           ES_DEV + (c + 1) * 128],
                                 rhs=es_t[:, ES_W1G:ES_W1G + 128],
                                 start=True, stop=True)
                std = sba.tile([128, 128], f32, tag="std")
                nc.scalar.copy(std[:], psd[:])
                nc.sync.dma_start(T_G_own[sl, 256:384], std[:])

        # ---------------- all-gather the big tables ----------------
        # T_L first: phase C (the long pole) only needs T_L.
        nc.gpsimd.collective_compute(
            "AllGather", mybir.AluOpType.bypass, replica_groups=RG,
            ins=[T_L_own.opt()], outs=[T_L.opt()])
        nc.gpsimd.collective_compute(
            "AllGather", mybir.AluOpType.bypass, replica_groups=RG,
            ins=[T_KVG_own.opt()], outs=[T_KVG.opt()])

        def gatherW(pool, tag, T_src, idx_off, c, W):
            """Gather 16 neighbor rows of width W for chunk c: [128,16,W]."""
            t = pool.tile([128, K, W], f32, tag=tag)
            isl = idx_t[:, idx_off + c * 128: idx_off + (c + 1) * 128]
            nc.gpsimd.dma_gather(t[:], T_src[:], isl, GS * 128, GS * 128, W,
                                 single_packet=False)
            return t

        # ---------------- phase B: kmax / vmax for own M block ----------
        with ExitStack() as pb:
            sbb = pb.enter_context(tc.tile_pool(name="sbb", bufs=2))
            for c in range(MB_CH):
                sl = slice(c * 128, (c + 1) * 128)
                kvg = gatherW(sbb, "kvg", T_KVG, IX_P, c, 256)
                km = sbb.tile([128, 128], f32, tag="km")
                nc.vector.tensor_reduce(
                    out=km[:], in_=kvg[:, :, 0:128].rearrange("p s a -> p a s"),
                    axis=AxX, op=Alu.max)
                vm = sbb.tile([128, 128], f32, tag="vm")
                nc.vector.tensor_reduce(
                    out=vm[:], in_=kvg[:, :, 128:256].rearrange("p s a -> p a s"),
                    axis=AxX, op=Alu.max)
                nc.sync.dma_start(T_G_own[sl, 0:128], km[:])
                nc.sync.dma_start(T_G_own[sl, 128:256], vm[:])

        nc.gpsimd.collective_compute(
            "AllGather", mybir.AluOpType.bypass, replica_groups=RG,
            ins=[T_G_own.opt()], outs=[T_G.opt()])

        # ---------------- attention chunk ----------------
        def attn_chunk(sb, psT, psP, c, T_pack, idx_off, u_own, q_own,
                       w2_ap, out_ap):
            sl = slice(c * 128, (c + 1) * 128)
            g = gatherW(sb, "g", T_pack, idx_off, c, 384)
            kg = g[:, :, 0:128]
            vg = g[:, :, 128:256]
            ug = g[:, :, 256:384]

            # qT for identity-matmul accumulation
            tq = psT.tile([128, 128], f32, tag="psT")
            nc.tensor.transpose(tq[:], q_own[:, sl], ident[:])
            qT = sb.tile([128, 128], f32, tag="qT")
            nc.scalar.copy(qT[:], tq[:])

            # pe layer-1: h = u_own (bcast over slots) - ug
            h = sb.tile([128, K, 128], f32, tag="h")
            nc.vector.tensor_tensor(out=h[:], in0=bcast_mid(u_own[:, sl], K),
                                    in1=ug, op=Alu.subtract)

            x = sb.tile([128, K, 128], f32, tag="x")
            wq = sb.tile([128, K, 128], f32, tag="wq")
            for g4 in range(K // 4):
                pp4 = psP.tile([128, 4, 128], f32, tag="pp4")
                for j in range(4):
                    s = g4 * 4 + j
                    tp = psT.tile([128, 128], f32, tag="psT")
                    nc.tensor.transpose(tp[:], h[:, s, :], ident[:])
                    hT = sb.tile([128, 128], f32, tag="hT")
                    nc.scalar.activation(hT[:], tp[:], Act.Relu)
                    nc.tensor.matmul(pp4[:, j, :], lhsT=hT[:], rhs=w2_ap,
                                     start=True, stop=False)
                    nc.tensor.matmul(pp4[:, j, :], lhsT=qT[:], rhs=ident[:],
                                     start=False, stop=True)
                gsl = slice(g4 * 4, g4 * 4 + 4)
                nc.vector.tensor_tensor(out=x[:, gsl, :], in0=pp4[:],
                                        in1=kg[:, gsl, :], op=Alu.subtract)
                nc.vector.tensor_tensor(out=wq[:, gsl, :], in0=vg[:, gsl, :],
                                        in1=pp4[:], op=Alu.add)

            # LN stats
            bn = sb.tile([128, K, 6], f32, tag="bn")
            for s in range(K):
                nc.vector.bn_stats(bn[:, s, :], x[:, s, :])
            ms = sb.tile([128, K], f32, tag="ms")
            nc.vector.tensor_tensor(out=ms[:], in0=bn[:, :, 1],
                                    in1=bn[:, :, 4], op=Alu.add)
            md = sb.tile([128, K], f32, tag="md")
            nc.vector.tensor_tensor(out=md[:], in0=bn[:, :, 1],
                                    in1=bn[:, :, 4], op=Alu.subtract)
            md2 = sb.tile([128, K], f32, tag="md2")
            nc.vector.tensor_tensor(out=md2[:], in0=md[:], in1=md[:],
                                    op=Alu.mult)
            cv = sb.tile([128, K], f32, tag="cv")
            nc.vector.tensor_tensor(out=cv[:], in0=bn[:, :, 2],
                                    in1=bn[:, :, 5], op=Alu.add)
            m2c = sb.tile([128, K], f32, tag="m2c")
            nc.vector.tensor_scalar_mul(m2c[:], md2[:], float(A) / 4.0)
            m2 = sb.tile([128, K], f32, tag="m2")
            nc.vector.tensor_tensor(out=m2[:], in0=cv[:], in1=m2c[:],
                                    op=Alu.add)
            var = sb.tile([128, K], f32, tag="var")
            nc.vector.tensor_scalar_mul(var[:], m2[:], 1.0 / A)
            std = sb.tile([128, K], f32, tag="std")
            nc.scalar.activation(std[:], var[:], Act.Sqrt, bias=eps_t[:])
            inv = sb.tile([128, K], f32, tag="inv")
            nc.vector.reciprocal(inv[:], std[:])
            asc = sb.tile([128, K], f32, tag="asc")
            nc.vector.tensor_scalar_mul(asc[:], inv[:], 1.0 / SCALE)
            nmean = sb.tile([128, K], f32, tag="nmean")
            nc.vector.tensor_scalar_mul(nmean[:], ms[:], -0.5)
            abi = sb.tile([128, K], f32, tag="abi")
            nc.vector.tensor_tensor(out=abi[:], in0=nmean[:], in1=asc[:],
                                    op=Alu.mult)

            # e = exp((x - mean) * inv / SCALE)
            e = sb.tile([128, K, 128], f32, tag="e")
            for s in range(K):
                nc.scalar.activation(e[:, s, :], x[:, s, :], Act.Exp,
                                     bias=abi[:, s:s + 1],
                                     scale=asc[:, s:s + 1])

            S0 = sb.tile([128, 128], f32, tag="S0")
            nc.vector.tensor_reduce(out=S0[:],
                                    in_=e[:].rearrange("p s a -> p a s"),
                                    axis=AxX, op=Alu.add)
            wp = sb.tile([128, K, 128], f32, tag="h")  # reuse h slots
            nc.vector.tensor_tensor(out=wp[:], in0=e[:], in1=wq[:],
                                    op=Alu.mult)
            S1 = sb.tile([128, 128], f32, tag="S1")
            nc.vector.tensor_reduce(out=S1[:],
                                    in_=wp[:].rearrange("p s a -> p a s"),
                                    axis=AxX, op=Alu.add)
            r0 = sb.tile([128, 128], f32, tag="r0")
            nc.vector.reciprocal(r0[:], S0[:])
            rat = sb.tile([128, 128], f32, tag="rat")
            nc.vector.tensor_tensor(out=rat[:], in0=S1[:], in1=r0[:],
                                    op=Alu.mult)
            nc.vector.tensor_tensor(out=out_ap, in0=rat[:], in1=q_own[:, sl],
                                    op=Alu.subtract)

        # ---------------- phase C: local attention ----------------
        with ExitStack() as pc:
            sbc = pc.enter_context(tc.tile_pool(name="sbc", bufs=2))
            psT = pc.enter_context(tc.tile_pool(name="psT", bufs=2,
                                                space="PSUM"))
            psP = pc.enter_context(tc.tile_pool(name="psP", bufs=2,
                                                space="PSUM"))
            for c in range(PC_CH):
                attn_chunk(sbc, psT, psP, c, T_L, IX_L,
                           uL_own, qL_own, wf_t[:, 0:128],
                           la_all[:, c * 128:(c + 1) * 128])

        # ---------------- phase D/E: global attention + proj -------------
        with ExitStack() as pd:
            sbd = pd.enter_context(tc.tile_pool(name="sbd", bufs=2))
            psT = pd.enter_context(tc.tile_pool(name="psT2", bufs=2,
                                                space="PSUM"))
            psP = pd.enter_context(tc.tile_pool(name="psP2", bufs=2,
                                                space="PSUM"))
            psH = pd.enter_context(tc.tile_pool(name="psH", bufs=2,
                                                space="PSUM"))
            for c in range(PC_CH):
                sl = slice(c * 128, (c + 1) * 128)
                ga = sbd.tile([128, 128], f32, tag="ga")
                attn_chunk(sbd, psT, psP, c, T_G, IX_G,
                           uG_own, qG_own, wf_t[:, 128:256], ga[:])
                # proj MLP on [la | ga] (bf16 weights)
                tl = psT.tile([128, 128], f32, tag="psT")
                nc.tensor.transpose(tl[:], la_all[:, sl], ident[:])
                laT = sbd.tile([128, 128], bf16, tag="laT")
                nc.scalar.copy(laT[:], tl[:])
                tg = psT.tile([128, 128], f32, tag="psT")
                nc.tensor.transpose(tg[:], ga[:], ident[:])
                gaT = sbd.tile([128, 128], bf16, tag="gaT")
                nc.scalar.copy(gaT[:], tg[:])
                psh = psH.tile([128, 256], f32, tag="psh")
                nc.tensor.matmul(psh[:], lhsT=laT[:],
                                 rhs=wb_t[:, WB_P1A:WB_P1A + 256],
                                 start=True, stop=False)
                nc.tensor.matmul(psh[:], lhsT=gaT[:],
                                 rhs=wb_t[:, WB_P1B:WB_P1B + 256],
                                 start=False, stop=True)
                hs = sbd.tile([128, 256], f32, tag="hs")
                nc.scalar.activation(hs[:], psh[:], Act.Relu)
                th0 = psT.tile([128, 128], f32, tag="psT")
                nc.tensor.transpose(th0[:], hs[:, 0:128], ident[:])
                hT0 = sbd.tile([128, 128], bf16, tag="hT0")
                nc.scalar.copy(hT0[:], th0[:])
                th1 = psT.tile([128, 128], f32, tag="psT")
                nc.tensor.transpose(th1[:], hs[:, 128:256], ident[:])
                hT1 = sbd.tile([128, 128], bf16, tag="hT1")
                nc.scalar.copy(hT1[:], th1[:])
                pso = psH.tile([128, 256], f32, tag="pso")
                nc.tensor.matmul(pso[:], lhsT=hT0[:],
                                 rhs=wb_t[:, WB_P2A:WB_P2A + 256],
                                 start=True, stop=False)
                nc.tensor.matmul(pso[:], lhsT=hT1[:],
                                 rhs=wb_t[:, WB_P2B:WB_P2B + 256],
                                 start=False, stop=True)
                oc = sbd.tile([128, 256], bf16, tag="oc")
                nc.vector.tensor_copy(oc[:], pso[:])
                nc.sync.dma_start(out_d[sl, :], oc[:])

    nc.compile()
    return nc


def _get_nc():
    if "nc" not in _CACHE:
        _CACHE["nc"] = _build()
    return _CACHE["nc"]


def _get_runner():
    """Build (once) a cached jitted shard_map callable for the NEFF."""
    if "runner" in _CACHE:
        return _CACHE["runner"]
    nc = _get_nc()
    import jax
    from jax.sharding import Mesh, PartitionSpec
    from jax.experimental.shard_map import shard_map
    from concourse import bass2jax, mybir

    bass2jax.install_neuronx_cc_hook()
    partition_name = (nc.partition_id_tensor.name
                      if nc.partition_id_tensor else None)
    in_names, out_names, out_avals, zero_templates = [], [], [], []
    for alloc in nc.m.functions[0].allocations:
        if not isinstance(alloc, mybir.MemoryLocationSet):
            continue
        name = alloc.memorylocations[0].name
        if alloc.kind == "ExternalInput":
            if name != partition_name:
                in_names.append(name)
        elif alloc.kind == "ExternalOutput":
            assert alloc.tensor_shape is not None and alloc.dtype is not None
            shape = tuple(alloc.tensor_shape)
            dt_np = mybir.dt.np(alloc.dtype)
            out_names.append(name)
            out_avals.append(jax.core.ShapedArray(shape, dt_np))
            zero_templates.append((shape, dt_np))
    n_params = len(in_names)
    n_outs = len(out_names)
    all_names = list(in_names) + list(out_names)
    if partition_name is not None:
        all_names.append(partition_name)
    donate = tuple(range(n_params, n_params + n_outs))

    def _body(*args):
        operands = list(args)
        if partition_name is not None:
            operands.append(bass2jax.partition_id_tensor())
        outs = bass2jax._bass_exec_p.bind(
            *operands,
            out_avals=tuple(out_avals),
            in_names=tuple(all_names),
            out_names=tuple(out_names),
            lowering_input_output_aliases=(),
            sim_require_finite=True,
            sim_require_nnan=True,
            nc=nc,
        )
        return tuple(outs)

    devices = jax.devices()[:NCORES]
    assert len(devices) == NCORES
    mesh = Mesh(np.asarray(devices), ("core",))
    in_specs = (PartitionSpec("core"),) * (n_params + n_outs)
    out_specs = (PartitionSpec("core"),) * n_outs
    fn = jax.jit(
        shard_map(_body, mesh=mesh, in_specs=in_specs, out_specs=out_specs,
                  check_rep=False),
        donate_argnums=donate, keep_unused=True)
    dbg = None
    if nc.dbg_addr is not None:
        assert not nc.dbg_callbacks
        dbg = nc.dbg_addr.name
    _CACHE["runner"] = (fn, in_names, zero_templates, dbg)
    return _CACHE["runner"]


def _remap_p(idx):
    """point index -> row in all-gathered T_L / T_KVG"""
    return (idx // NPC) * PCPAD + (idx % NPC)


def _remap_m(idx):
    """down-point index -> row in all-gathered T_G"""
    return (idx // MPC) * MBPAD + (idx % MPC)


def _wrap(idx2d):
    """[rows (mult of 128), 16] int -> [16, rows] i16 dma_gather order."""
    nch = idx2d.shape[0] // 128
    a = idx2d.reshape(nch, 128, K).transpose(0, 2, 1).reshape(nch, 128 * K)
    w = a.reshape(nch, 128, 16).transpose(2, 0, 1).reshape(16, nch * 128)
    return np.ascontiguousarray(w.astype(np.int16))


def _pad_rows(x, rows):
    out = np.zeros((rows,) + x.shape[1:], dtype=x.dtype)
    out[: x.shape[0]] = x
    return out


def kernel(**inputs):
    import ml_dtypes
    bf16 = ml_dtypes.bfloat16

    events = np.asarray(inputs["events"], np.float32)
    features = np.asarray(inputs["features"], np.float32)
    local_idx = np.asarray(inputs["local_idx"], np.int32)
    down_idx = np.asarray(inputs["down_idx"], np.int32)
    pair_idx = np.asarray(inputs["pair_idx"], np.int32)
    inv_pair_idx = np.asarray(inputs["inv_pair_idx"], np.int32)

    for nm in ("local_qkv_b", "local_pe_b1", "local_pe_b2", "local_fc_b",
               "global_qkv_b", "global_pe_b1", "global_pe_b2", "global_fc_b",
               "proj_b1", "proj_b2"):
        assert np.abs(np.asarray(inputs[nm])).max() == 0.0, f"{nm} nonzero"
    for nm in ("local_fc_g", "global_fc_g"):
        assert np.abs(np.asarray(inputs[nm]) - 1.0).max() == 0.0

    lw = np.asarray(inputs["local_qkv_w"], np.float32)
    gw = np.asarray(inputs["global_qkv_w"], np.float32)
    qL, kL, vL = lw[:, 0:A], lw[:, A:2 * A], lw[:, 2 * A:3 * A]
    qG, kG, vG = gw[:, 0:A], gw[:, A:2 * A], gw[:, 2 * A:3 * A]
    Wkv = np.concatenate([kL, vL, kG, vG], axis=1)          # [256, 512]
    Wq = np.concatenate([qL, qG], axis=1)                   # [256, 256]
    w1L = np.asarray(inputs["local_pe_w1"], np.float32)
    w1G = np.asarray(inputs["global_pe_w1"], np.float32)
    Wu = np.concatenate([w1L, w1G], axis=1)                 # [4, 256]
    pw1 = np.asarray(inputs["proj_w1"], np.float32)
    pw2 = np.asarray(inputs["proj_w2"], np.float32)

    WBh = np.concatenate(
        [Wkv[0:128], Wkv[128:256], Wq[0:128], Wq[128:256],
         pw1[0:128], pw1[128:256], pw2[0:128], pw2[128:256]],
        axis=1).astype(bf16)                                # [128, 2560]
    WFh = np.ascontiguousarray(np.concatenate(
        [np.asarray(inputs["local_pe_w2"], np.float32),
         np.asarray(inputs["global_pe_w2"], np.float32)], axis=1))

    dev_events = events[down_idx]                           # [M, 4]
    lidx_r = _remap_p(local_idx)                            # [N, 16]
    gidx_r = _remap_m(inv_pair_idx)                         # [N, 16]
    pidx_r = _remap_p(pair_idx)                             # [M, 16]
    feat_bf = features.astype(bf16)

    # build the global (axis-0 concatenated) shard_map inputs directly
    featR_g = np.zeros((NCORES * PCPAD, DIM), bf16)
    ES_g = np.zeros((NCORES * 4, ES_COLS), np.float32)
    IDX_g = np.empty((NCORES * 16, IX_COLS), np.int16)
    for core in range(NCORES):
        r0 = core * NPC
        m0 = core * MPC
        featR_g[core * PCPAD:core * PCPAD + NPC] = feat_bf[r0:r0 + NPC]
        es = ES_g[core * 4:(core + 1) * 4]
        es[:, :NPC] = events[r0:r0 + NPC].T
        mde = dev_events[m0:m0 + MPC]
        es[:, ES_DEV:ES_DEV + mde.shape[0]] = mde.T
        es[:, ES_WU:ES_WU + 256] = Wu
        idxb = IDX_g[core * 16:(core + 1) * 16]
        idxb[:, IX_L:IX_L + PCPAD] = _wrap(
            _pad_rows(lidx_r[r0:r0 + NPC], PCPAD))
        idxb[:, IX_G:IX_G + PCPAD] = _wrap(
            _pad_rows(gidx_r[r0:r0 + NPC], PCPAD))
        idxb[:, IX_P:IX_P + MBPAD] = _wrap(
            _pad_rows(pidx_r[m0:m0 + MPC], MBPAD))

    from concourse._compat import axon_active
    if not axon_active():
        # native-NRT fallback: go through run_bass_kernel_spmd
        from concourse.bass_utils import run_bass_kernel_spmd
        in_maps = []
        for core in range(NCORES):
            in_maps.append({
                "featR": featR_g[core * PCPAD:(core + 1) * PCPAD],
                "ES": ES_g[core * 4:(core + 1) * 4],
                "IDX": IDX_g[core * 16:(core + 1) * 16],
                "WBs": WBh[core * 16:(core + 1) * 16],
                "WFs": WFh[core * 16:(core + 1) * 16],
            })
        res = run_bass_kernel_spmd(_get_nc(), in_maps,
                                   core_ids=list(range(NCORES)))
        out = np.stack([np.asarray(res.results[i]["out"][:NPC], np.float32)
                        for i in range(NCORES)], axis=0)
        return np.ascontiguousarray(out.reshape(N, DIM)).astype(np.float32)

    fn, in_names, zero_templates, dbg = _get_runner()
    globals_by_name = {
        "featR": featR_g,
        "ES": ES_g,
        "IDX": IDX_g,
        "WBs": WBh,      # [128, 2560] == concat of per-core [16, 2560]
        "WFs": WFh,      # [128, 256]
    }
    if dbg is not None:
        globals_by_name[dbg] = np.zeros((NCORES, 2), np.uint32)
    concat_in = [globals_by_name[nm] for nm in in_names]
    # The kernel writes every element of the output, so the donated
    # "zero" buffers don't need to hold zeros: recycle the previous
    # call's device-resident outputs to skip re-uploading them.
    prev = _CACHE.get("prev_outs")
    if prev is None:
        prev = [
            np.zeros((NCORES * s[0],) + tuple(s[1:]), dt)
            for (s, dt) in zero_templates
        ]
    _CACHE["last_args"] = (concat_in, zero_templates)
    outs = fn(*concat_in, *prev)
    res = np.asarray(outs[0])                               # [8*2560, 256]
    _CACHE["prev_outs"] = list(outs)
    out = res.reshape(NCORES, PCPAD, DIM)[:, :NPC]
    return np.ascontiguousarray(out.reshape(N, DIM)).astype(np.float32)
